# revision 16
# baseline (speedup 1.0000x reference)
"""GCMCGraphConv Bass kernel for 8 TRN2 NeuronCores.

Computes: h = ci * segment_sum((weight * cj)[src], dst)  for a random
graph with N=100000 nodes, F=128 features, E=1600000 edges.

Strategy (1D dst-partitioning, v4):
  - core c owns dst rows [c*12500, (c+1)*12500)
  - host partitions edges by dst owner, groups by (dst block, src
    segment), pads each (block, segment) run to K_s chunks of 128
    edges (uniform across blocks/cores so the SPMD program is static)
  - device phase 1 (prescale): wsc = bf16(weight * cj), written to 4
    internal DRAM segments of 25600 rows (so gather indices fit int16)
  - device phase 2: per src segment the edge-source rows are fetched
    by dma_gather ucode instructions of 1024 indices (8 chunks) each;
    per (block, segment) a batched is_equal one-hot of dst_local
    (ACT/DVE) and per chunk a bf16 matmul accumulate the segment sum
    in PSUM; ci scaling and an output DMA finish each block
"""

import os
import sys

import numpy as np

sys.path.insert(0, "/opt/trn_rl_repo")

from concourse import bacc, bass, mybir  # noqa: E402
import concourse.tile as tile  # noqa: E402
from concourse.bass_utils import run_bass_kernel_spmd  # noqa: E402

N_NODES = 100000
FEAT = 128
N_CORES = 8
DST_PER_CORE = N_NODES // N_CORES  # 12500
P = 128
N_BLOCKS = (DST_PER_CORE + P - 1) // P  # 98
DST_PAD = N_BLOCKS * P  # 12544

SEG = 4
SEG_ROWS = 25600  # multiple of SUPER; int16-addressable
N_PAD = SEG * SEG_ROWS  # 102400
SUPER = 1024  # convert-pass superblock rows
PIECE = 8  # chunks per dma_gather instruction (1024 idx ring limit)

LAST_EXEC_NS = None


def _ensure_ntff_hook():
    """Shim antenv.axon_hooks if the image's antenv predates it."""
    import types

    try:
        from antenv.axon_hooks import get_axon_ntff_profile_hook  # noqa: F401

        return
    except ImportError:
        pass
    try:
        import antenv

        mod = types.ModuleType("antenv.axon_hooks")
        _hook = [None]
        mod.set_axon_ntff_profile_hook = lambda h: _hook.__setitem__(0, h)
        mod.get_axon_ntff_profile_hook = lambda: _hook[0]
        antenv.axon_hooks = mod
        sys.modules["antenv.axon_hooks"] = mod
        from trn_agent_boot.trn_boot import _ntff_profile_via_ctypes

        mod.set_axon_ntff_profile_hook(
            _ntff_profile_via_ctypes("/opt/axon/libaxon_pjrt.so")
        )
    except Exception:
        pass


def _build_program(k_s: int) -> bass.Bass:
    """One SPMD program; every core runs it on its own edge shard."""
    nc = bacc.Bacc(num_swdge_queues=4)
    f32 = mybir.dt.float32
    bf16 = mybir.dt.bfloat16
    i32 = mybir.dt.int32
    i16 = mybir.dt.int16

    n_super = N_PAD // SUPER
    blk_chunks = SEG * k_s  # chunks per dst block
    ncols = N_BLOCKS * blk_chunks
    seg_chunks = N_BLOCKS * k_s  # chunks per segment
    n_pieces = (seg_chunks + PIECE - 1) // PIECE
    idxcols_per_piece = PIECE * P // 16  # 64

    w_d = nc.declare_dram_parameter("w", [N_PAD, FEAT], f32, isOutput=False)
    cjb_d = nc.declare_dram_parameter("cjb", [P, N_PAD // P], f32, isOutput=False)
    gidx_d = nc.declare_dram_parameter(
        "gidx", [P, SEG * n_pieces * idxcols_per_piece], i16, isOutput=False
    )
    dstloc_d = nc.declare_dram_parameter("dstloc", [P, ncols], bf16, isOutput=False)
    cib_d = nc.declare_dram_parameter("cib", [P, N_BLOCKS], f32, isOutput=False)
    h_d = nc.declare_dram_parameter("h", [DST_PAD, FEAT], f32, isOutput=True)

    with tile.TileContext(nc) as tc:
        with (
            tc.tile_pool(name="wscp", bufs=1, space="DRAM") as wscp,
            tc.tile_pool(name="meta", bufs=1) as meta,
            tc.tile_pool(name="conv", bufs=3) as conv,
            tc.tile_pool(name="gather", bufs=5) as gpool,
            tc.tile_pool(name="work", bufs=4) as work,
            tc.tile_pool(name="out", bufs=3) as opool,
            tc.tile_pool(name="psum", bufs=2, space="PSUM") as psum,
        ):
            wsc = [
                wscp.tile([SEG_ROWS, FEAT], bf16, tag=f"wsc{s}", name=f"wsc{s}")
                for s in range(SEG)
            ]

            gidx = meta.tile([P, SEG * n_pieces * idxcols_per_piece], i16)
            dstloc = meta.tile([P, ncols], bf16)
            cib = meta.tile([P, N_BLOCKS], f32)
            cjb = meta.tile([P, N_PAD // P], f32)
            nc.sync.dma_start(out=gidx[:], in_=gidx_d[:])
            nc.sync.dma_start(out=dstloc[:], in_=dstloc_d[:])
            nc.sync.dma_start(out=cib[:], in_=cib_d[:])
            nc.sync.dma_start(out=cjb[:], in_=cjb_d[:])

            # iota5[p, j*128 + f] = f  (int32)
            iota5i = meta.tile([P, k_s * P], i32)
            nc.gpsimd.iota(
                iota5i[:], pattern=[[0, k_s], [1, P]], base=0, channel_multiplier=0
            )
            iota5 = meta.tile([P, k_s * P], bf16)
            nc.vector.tensor_copy(out=iota5[:], in_=iota5i[:])

            # phase 1: wsc[seg] = bf16(w * cj), 1024-row superblocks
            spb = SUPER // P  # 8 column-groups per superblock
            sbs_per_seg = SEG_ROWS // SUPER
            for sb in range(n_super):
                wt = conv.tile([P, SUPER], f32, tag="wt")
                win = w_d[sb * SUPER : (sb + 1) * SUPER, :].rearrange(
                    "(g p) f -> p g f", p=P
                )
                nc.sync.dma_start(
                    out=wt[:].rearrange("p (g f) -> p g f", f=FEAT), in_=win
                )
                ws = conv.tile([P, SUPER], bf16, tag="ws")
                nc.vector.tensor_tensor(
                    out=ws[:].rearrange("p (g f) -> p g f", f=FEAT),
                    in0=wt[:].rearrange("p (g f) -> p g f", f=FEAT),
                    in1=cjb[:, sb * spb : (sb + 1) * spb].to_broadcast([P, spb, FEAT]),
                    op=mybir.AluOpType.mult,
                )
                s = sb // sbs_per_seg
                lb = sb % sbs_per_seg
                wout = wsc[s][lb * SUPER : (lb + 1) * SUPER, :].rearrange(
                    "(g p) f -> p g f", p=P
                )
                nc.sync.dma_start(
                    out=wout, in_=ws[:].rearrange("p (g f) -> p g f", f=FEAT)
                )

            # phase 2: issue all gathers (Tile paces them via pool bufs),
            # then per block: one-hot + matmul chunks, ci scale, store.
            gts: list[list] = [[] for _ in range(SEG)]
            for pc in range(n_pieces):
                for s in range(SEG):
                    nchunk = min(PIECE, seg_chunks - pc * PIECE)
                    gt = gpool.tile([P, PIECE * FEAT], bf16, tag=f"gw{s}")
                    inst = s * n_pieces + pc
                    nc.gpsimd.dma_gather(
                        gt[:, : nchunk * FEAT].rearrange("p (m f) -> p m f", f=FEAT),
                        wsc[s][:],
                        gidx[
                            :,
                            inst * idxcols_per_piece : inst * idxcols_per_piece
                            + nchunk * P // 16,
                        ],
                        nchunk * P,
                        nchunk * P,
                        FEAT,
                        queue_num=s,
                    )
                    gts[s].append(gt)

            for b in range(N_BLOCKS):
                acc = psum.tile([P, FEAT], f32, tag="acc")
                ci_i = 0
                for s in range(SEG):
                    onehot = work.tile([P, k_s * P], bf16, tag="onehot")
                    nc.any.tensor_tensor(
                        out=onehot[:].rearrange("p (m f) -> p m f", f=P),
                        in0=dstloc[
                            :, b * blk_chunks + s * k_s : b * blk_chunks + (s + 1) * k_s
                        ].to_broadcast([P, k_s, P]),
                        in1=iota5[:].rearrange("p (m f) -> p m f", f=P),
                        op=mybir.AluOpType.is_equal,
                    )
                    for k in range(k_s):
                        q = b * k_s + k  # global chunk index within segment
                        gt = gts[s][q // PIECE]
                        off = q % PIECE
                        nc.tensor.matmul(
                            out=acc[:],
                            lhsT=onehot[:, k * P : (k + 1) * P],
                            rhs=gt[:, off * FEAT : (off + 1) * FEAT],
                            start=(ci_i == 0),
                            stop=(ci_i == blk_chunks - 1),
                        )
                        ci_i += 1
                ho = opool.tile([P, FEAT], f32, tag="ho")
                nc.vector.tensor_tensor(
                    out=ho[:],
                    in0=acc[:],
                    in1=cib[:, b : b + 1].to_broadcast([P, FEAT]),
                    op=mybir.AluOpType.mult,
                )
                nc.sync.dma_start(out=h_d[b * P : (b + 1) * P, :], in_=ho[:])
    return nc


def _prep_inputs(weight, cj, ci, src, dst):
    """Partition edges by dst owner; build per-core metadata arrays."""
    order = np.argsort(dst, kind="stable")
    ds = dst[order].astype(np.int64)
    ss = src[order].astype(np.int64)
    core_bounds = np.searchsorted(ds, np.arange(N_CORES + 1) * DST_PER_CORE)

    cores = []
    k_s = 1
    for c in range(N_CORES):
        a, b = core_bounds[c], core_bounds[c + 1]
        d_local = ds[a:b] - c * DST_PER_CORE
        s_c = ss[a:b]
        seg = s_c // SEG_ROWS
        block = d_local // P
        o2 = np.lexsort((seg, block))
        d_local, s_c, seg, block = d_local[o2], s_c[o2], seg[o2], block[o2]
        bs = block * SEG + seg
        counts = np.bincount(bs, minlength=N_BLOCKS * SEG)
        k_s = max(k_s, int(np.ceil(counts.max() / P)))
        cores.append((d_local, s_c, bs, counts))

    blk_chunks = SEG * k_s
    ncols = N_BLOCKS * blk_chunks
    seg_chunks = N_BLOCKS * k_s
    n_pieces = (seg_chunks + PIECE - 1) // PIECE
    idxcols_per_piece = PIECE * P // 16

    cj_flat = cj.reshape(-1).astype(np.float32)
    ci_flat = ci.reshape(-1).astype(np.float32)

    w_pad = np.zeros((N_PAD, FEAT), dtype=np.float32)
    w_pad[:N_NODES] = weight
    cj_pad = np.zeros(N_PAD, dtype=np.float32)
    cj_pad[:N_NODES] = cj_flat
    cjb = cj_pad.reshape(N_PAD // P, P).T.copy()

    in_maps = []
    for c in range(N_CORES):
        d_local, s_c, bs, counts = cores[c]
        starts = np.zeros(N_BLOCKS * SEG, dtype=np.int64)
        starts[1:] = np.cumsum(counts)[:-1]
        wbi = np.arange(len(d_local)) - starts[bs]  # index within (block, seg) run
        kk = wbi // P
        pp = wbi % P
        col = (bs // SEG) * blk_chunks + (bs % SEG) * k_s + kk

        import ml_dtypes

        dstloc = np.full((P, ncols), -1, dtype=ml_dtypes.bfloat16)
        dstloc[pp, col] = (d_local % P).astype(ml_dtypes.bfloat16)
        srcloc = np.zeros((P, ncols), dtype=np.int16)
        srcloc[pp, col] = (s_c % SEG_ROWS).astype(np.int16)

        # gather index arrays: per (seg, piece) instruction, idx j at
        # [16*grp + j%16, j//16]; j = (chunk_within_piece*128 + p),
        # chunk q (= b*k_s + k) of segment s is piece q//PIECE.
        gidx = np.zeros((P, SEG * n_pieces * idxcols_per_piece), dtype=np.int16)
        for s in range(SEG):
            # [P, seg_chunks] source-local indices for this segment in
            # chunk order q = b*k_s + k  -> col = b*blk_chunks + s*k_s + k
            cols = (
                (np.arange(N_BLOCKS)[:, None] * blk_chunks)
                + s * k_s
                + np.arange(k_s)[None, :]
            ).reshape(-1)
            segsrc = srcloc[:, cols]  # [P, seg_chunks]
            vals = segsrc.T.reshape(-1)  # j = q*128 + p
            vals = np.pad(vals, (0, n_pieces * PIECE * P - len(vals)))
            block16 = vals.reshape(n_pieces * idxcols_per_piece, 16).T  # [16, cols]
            gidx[
                :, s * n_pieces * idxcols_per_piece : (s + 1) * n_pieces * idxcols_per_piece
            ] = np.tile(block16, (8, 1))

        ci_pad = np.zeros(DST_PAD, dtype=np.float32)
        ci_pad[:DST_PER_CORE] = ci_flat[c * DST_PER_CORE : (c + 1) * DST_PER_CORE]
        cib = ci_pad.reshape(N_BLOCKS, P).T.copy()

        in_maps.append(
            {
                "w": w_pad,
                "cjb": cjb,
                "gidx": gidx,
                "dstloc": dstloc,
                "cib": cib,
            }
        )
    return in_maps, k_s


def _maybe_enable_ldw_opt():
    if not int(os.environ.get("KERNEL_LDW", "1")):
        return
    import concourse.bass_utils as _bu

    if getattr(_bu, "_ldw_patched", False):
        return
    _orig = _bu.run_command

    def _patched(argv, **kw):
        argv = [
            "--enable-ldw-opt=true" if a == "--enable-ldw-opt=false" else a
            for a in argv
        ]
        return _orig(argv, **kw)

    _bu.run_command = _patched
    _bu._ldw_patched = True


def kernel(weight, cj, ci, src, dst):
    global LAST_EXEC_NS
    _maybe_enable_ldw_opt()
    weight = np.asarray(weight, dtype=np.float32)
    cj = np.asarray(cj, dtype=np.float32)
    ci = np.asarray(ci, dtype=np.float32)
    src = np.asarray(src, dtype=np.int32)
    dst = np.asarray(dst, dtype=np.int32)

    in_maps, k_s = _prep_inputs(weight, cj, ci, src, dst)
    nc = _build_program(k_s)
    nc.finalize()
    trace = bool(int(os.environ.get("KERNEL_TRACE", "0")))
    if trace:
        _ensure_ntff_hook()
    try:
        res = run_bass_kernel_spmd(
            nc, in_maps, core_ids=list(range(N_CORES)), trace=trace
        )
    except Exception:
        if not trace:
            raise
        res = run_bass_kernel_spmd(
            nc, in_maps, core_ids=list(range(N_CORES)), trace=False
        )
    LAST_EXEC_NS = res.exec_time_ns
    out = np.concatenate(
        [res.results[c]["h"][:DST_PER_CORE] for c in range(N_CORES)], axis=0
    )
    return out.astype(np.float32)


# revision 17
# speedup vs baseline: 1.0337x; 1.0337x over previous
"""GCMCGraphConv Bass kernel for 8 TRN2 NeuronCores.

Computes: h = ci * segment_sum((weight * cj)[src], dst)  for a random
graph with N=100000 nodes, F=128 features, E=1600000 edges.

Strategy (1D dst-partitioning, v4):
  - core c owns dst rows [c*12500, (c+1)*12500)
  - host partitions edges by dst owner, groups by (dst block, src
    segment), pads each (block, segment) run to K_s chunks of 128
    edges (uniform across blocks/cores so the SPMD program is static)
  - device phase 1 (prescale): wsc = bf16(weight * cj), written to 4
    internal DRAM segments of 25600 rows (so gather indices fit int16)
  - device phase 2: per src segment the edge-source rows are fetched
    by dma_gather ucode instructions of 1024 indices (8 chunks) each;
    per (block, segment) a batched is_equal one-hot of dst_local
    (ACT/DVE) and per chunk a bf16 matmul accumulate the segment sum
    in PSUM; ci scaling and an output DMA finish each block
"""

import os
import sys

import numpy as np

sys.path.insert(0, "/opt/trn_rl_repo")

from concourse import bacc, bass, mybir  # noqa: E402
import concourse.tile as tile  # noqa: E402
from concourse.bass_utils import run_bass_kernel_spmd  # noqa: E402

N_NODES = 100000
FEAT = 128
N_CORES = 8
DST_PER_CORE = N_NODES // N_CORES  # 12500
P = 128
N_BLOCKS = (DST_PER_CORE + P - 1) // P  # 98
DST_PAD = N_BLOCKS * P  # 12544

SEG = 4
SEG_ROWS = 25600  # multiple of SUPER; int16-addressable
N_PAD = SEG * SEG_ROWS  # 102400
SUPER = 1024  # convert-pass superblock rows
PIECE = 8  # chunks per dma_gather instruction (1024 idx ring limit)

LAST_EXEC_NS = None


def _ensure_ntff_hook():
    """Shim antenv.axon_hooks if the image's antenv predates it."""
    import types

    try:
        from antenv.axon_hooks import get_axon_ntff_profile_hook  # noqa: F401

        return
    except ImportError:
        pass
    try:
        import antenv

        mod = types.ModuleType("antenv.axon_hooks")
        _hook = [None]
        mod.set_axon_ntff_profile_hook = lambda h: _hook.__setitem__(0, h)
        mod.get_axon_ntff_profile_hook = lambda: _hook[0]
        antenv.axon_hooks = mod
        sys.modules["antenv.axon_hooks"] = mod
        from trn_agent_boot.trn_boot import _ntff_profile_via_ctypes

        mod.set_axon_ntff_profile_hook(
            _ntff_profile_via_ctypes("/opt/axon/libaxon_pjrt.so")
        )
    except Exception:
        pass


def _build_program(k_s: int) -> bass.Bass:
    """One SPMD program; every core runs it on its own edge shard."""
    nc = bacc.Bacc(num_swdge_queues=4)
    f32 = mybir.dt.float32
    bf16 = mybir.dt.bfloat16
    i32 = mybir.dt.int32
    i16 = mybir.dt.int16

    n_super = N_PAD // SUPER
    blk_chunks = SEG * k_s  # chunks per dst block
    ncols = N_BLOCKS * blk_chunks
    seg_chunks = N_BLOCKS * k_s  # chunks per segment
    n_pieces = (seg_chunks + PIECE - 1) // PIECE
    idxcols_per_piece = PIECE * P // 16  # 64

    w_d = nc.declare_dram_parameter("w", [N_PAD, FEAT], f32, isOutput=False)
    cjb_d = nc.declare_dram_parameter("cjb", [P, N_PAD // P], f32, isOutput=False)
    gidx_d = nc.declare_dram_parameter(
        "gidx", [P, SEG * n_pieces * idxcols_per_piece], i16, isOutput=False
    )
    dstloc_d = nc.declare_dram_parameter("dstloc", [P, ncols], bf16, isOutput=False)
    cib_d = nc.declare_dram_parameter("cib", [P, N_BLOCKS], f32, isOutput=False)
    h_d = nc.declare_dram_parameter("h", [DST_PAD, FEAT], f32, isOutput=True)

    with tile.TileContext(nc) as tc:
        with (
            tc.tile_pool(name="wscp", bufs=1, space="DRAM") as wscp,
            tc.tile_pool(name="meta", bufs=1) as meta,
            tc.tile_pool(name="conv", bufs=3) as conv,
            tc.tile_pool(name="gather", bufs=5) as gpool,
            tc.tile_pool(name="work", bufs=4) as work,
            tc.tile_pool(name="out", bufs=3) as opool,
            tc.tile_pool(name="psum", bufs=2, space="PSUM") as psum,
        ):
            wsc = [
                wscp.tile([SEG_ROWS, FEAT], bf16, tag=f"wsc{s}", name=f"wsc{s}")
                for s in range(SEG)
            ]

            gidx = meta.tile([P, SEG * n_pieces * idxcols_per_piece], i16)
            dstloc = meta.tile([P, ncols], bf16)
            cib = meta.tile([P, N_BLOCKS], f32)
            cjb = meta.tile([P, N_PAD // P], f32)
            nc.sync.dma_start(out=gidx[:], in_=gidx_d[:])
            nc.sync.dma_start(out=dstloc[:], in_=dstloc_d[:])
            nc.sync.dma_start(out=cib[:], in_=cib_d[:])
            nc.sync.dma_start(out=cjb[:], in_=cjb_d[:])

            # iota5[p, j*128 + f] = f  (int32)
            iota5i = meta.tile([P, k_s * P], i32)
            nc.gpsimd.iota(
                iota5i[:], pattern=[[0, k_s], [1, P]], base=0, channel_multiplier=0
            )
            iota5 = meta.tile([P, k_s * P], bf16)
            nc.vector.tensor_copy(out=iota5[:], in_=iota5i[:])

            # phase 1: wsc[seg] = bf16(w * cj), 1024-row superblocks
            spb = SUPER // P  # 8 column-groups per superblock
            sbs_per_seg = SEG_ROWS // SUPER
            for sb in range(n_super):
                wt = conv.tile([P, SUPER], f32, tag="wt")
                win = w_d[sb * SUPER : (sb + 1) * SUPER, :].rearrange(
                    "(g p) f -> p g f", p=P
                )
                nc.sync.dma_start(
                    out=wt[:].rearrange("p (g f) -> p g f", f=FEAT), in_=win
                )
                ws = conv.tile([P, SUPER], bf16, tag="ws")
                nc.vector.tensor_tensor(
                    out=ws[:].rearrange("p (g f) -> p g f", f=FEAT),
                    in0=wt[:].rearrange("p (g f) -> p g f", f=FEAT),
                    in1=cjb[:, sb * spb : (sb + 1) * spb].to_broadcast([P, spb, FEAT]),
                    op=mybir.AluOpType.mult,
                )
                s = sb // sbs_per_seg
                lb = sb % sbs_per_seg
                wout = wsc[s][lb * SUPER : (lb + 1) * SUPER, :].rearrange(
                    "(g p) f -> p g f", p=P
                )
                nc.sync.dma_start(
                    out=wout, in_=ws[:].rearrange("p (g f) -> p g f", f=FEAT)
                )

            # phase 2: issue all gathers (Tile paces them via pool bufs),
            # then per block: one-hot + matmul chunks, ci scale, store.
            gts: list[list] = [[] for _ in range(SEG)]
            for pc in range(n_pieces):
                for s in range(SEG):
                    nchunk = min(PIECE, seg_chunks - pc * PIECE)
                    gt = gpool.tile([P, PIECE * FEAT], bf16, tag=f"gw{s}")
                    inst = s * n_pieces + pc
                    nc.gpsimd.dma_gather(
                        gt[:, : nchunk * FEAT].rearrange("p (m f) -> p m f", f=FEAT),
                        wsc[s][:],
                        gidx[
                            :,
                            inst * idxcols_per_piece : inst * idxcols_per_piece
                            + nchunk * P // 16,
                        ],
                        nchunk * P,
                        nchunk * P,
                        FEAT,
                        queue_num=s,
                    )
                    gts[s].append(gt)

            for b in range(N_BLOCKS):
                acc = psum.tile([P, FEAT], f32, tag="acc")
                ci_i = 0
                for s in range(SEG):
                    onehot = work.tile([P, k_s * P], bf16, tag="onehot")
                    nc.any.tensor_tensor(
                        out=onehot[:].rearrange("p (m f) -> p m f", f=P),
                        in0=dstloc[
                            :, b * blk_chunks + s * k_s : b * blk_chunks + (s + 1) * k_s
                        ].to_broadcast([P, k_s, P]),
                        in1=iota5[:].rearrange("p (m f) -> p m f", f=P),
                        op=mybir.AluOpType.is_equal,
                    )
                    for k in range(k_s):
                        q = b * k_s + k  # global chunk index within segment
                        gt = gts[s][q // PIECE]
                        off = q % PIECE
                        nc.tensor.matmul(
                            out=acc[:],
                            lhsT=onehot[:, k * P : (k + 1) * P],
                            rhs=gt[:, off * FEAT : (off + 1) * FEAT],
                            start=(ci_i == 0),
                            stop=(ci_i == blk_chunks - 1),
                        )
                        ci_i += 1
                ho = opool.tile([P, FEAT], f32, tag="ho")
                nc.vector.tensor_tensor(
                    out=ho[:],
                    in0=acc[:],
                    in1=cib[:, b : b + 1].to_broadcast([P, FEAT]),
                    op=mybir.AluOpType.mult,
                )
                nc.sync.dma_start(out=h_d[b * P : (b + 1) * P, :], in_=ho[:])
    return nc


def _prep_inputs(weight, cj, ci, src, dst):
    """Partition edges by dst owner; build per-core metadata arrays."""
    order = np.argsort(dst, kind="stable")
    ds = dst[order].astype(np.int64)
    ss = src[order].astype(np.int64)
    core_bounds = np.searchsorted(ds, np.arange(N_CORES + 1) * DST_PER_CORE)

    cores = []
    k_s = 1
    for c in range(N_CORES):
        a, b = core_bounds[c], core_bounds[c + 1]
        d_local = ds[a:b] - c * DST_PER_CORE
        s_c = ss[a:b]
        seg = s_c // SEG_ROWS
        block = d_local // P
        o2 = np.lexsort((seg, block))
        d_local, s_c, seg, block = d_local[o2], s_c[o2], seg[o2], block[o2]
        bs = block * SEG + seg
        counts = np.bincount(bs, minlength=N_BLOCKS * SEG)
        k_s = max(k_s, int(np.ceil(counts.max() / P)))
        cores.append((d_local, s_c, bs, counts))

    blk_chunks = SEG * k_s
    ncols = N_BLOCKS * blk_chunks
    seg_chunks = N_BLOCKS * k_s
    n_pieces = (seg_chunks + PIECE - 1) // PIECE
    idxcols_per_piece = PIECE * P // 16

    cj_flat = cj.reshape(-1).astype(np.float32)
    ci_flat = ci.reshape(-1).astype(np.float32)

    w_pad = np.zeros((N_PAD, FEAT), dtype=np.float32)
    w_pad[:N_NODES] = weight
    cj_pad = np.zeros(N_PAD, dtype=np.float32)
    cj_pad[:N_NODES] = cj_flat
    cjb = cj_pad.reshape(N_PAD // P, P).T.copy()

    in_maps = []
    for c in range(N_CORES):
        d_local, s_c, bs, counts = cores[c]
        starts = np.zeros(N_BLOCKS * SEG, dtype=np.int64)
        starts[1:] = np.cumsum(counts)[:-1]
        wbi = np.arange(len(d_local)) - starts[bs]  # index within (block, seg) run
        kk = wbi // P
        pp = wbi % P
        col = (bs // SEG) * blk_chunks + (bs % SEG) * k_s + kk

        import ml_dtypes

        dstloc = np.full((P, ncols), -1, dtype=ml_dtypes.bfloat16)
        dstloc[pp, col] = (d_local % P).astype(ml_dtypes.bfloat16)
        srcloc = np.zeros((P, ncols), dtype=np.int16)
        srcloc[pp, col] = (s_c % SEG_ROWS).astype(np.int16)

        # gather index arrays: per (seg, piece) instruction, idx j at
        # [16*grp + j%16, j//16]; j = (chunk_within_piece*128 + p),
        # chunk q (= b*k_s + k) of segment s is piece q//PIECE.
        gidx = np.zeros((P, SEG * n_pieces * idxcols_per_piece), dtype=np.int16)
        for s in range(SEG):
            # [P, seg_chunks] source-local indices for this segment in
            # chunk order q = b*k_s + k  -> col = b*blk_chunks + s*k_s + k
            cols = (
                (np.arange(N_BLOCKS)[:, None] * blk_chunks)
                + s * k_s
                + np.arange(k_s)[None, :]
            ).reshape(-1)
            segsrc = srcloc[:, cols]  # [P, seg_chunks]
            vals = segsrc.T.reshape(-1)  # j = q*128 + p
            vals = np.pad(vals, (0, n_pieces * PIECE * P - len(vals)))
            block16 = vals.reshape(n_pieces * idxcols_per_piece, 16).T  # [16, cols]
            gidx[
                :, s * n_pieces * idxcols_per_piece : (s + 1) * n_pieces * idxcols_per_piece
            ] = np.tile(block16, (8, 1))

        ci_pad = np.zeros(DST_PAD, dtype=np.float32)
        ci_pad[:DST_PER_CORE] = ci_flat[c * DST_PER_CORE : (c + 1) * DST_PER_CORE]
        cib = ci_pad.reshape(N_BLOCKS, P).T.copy()

        in_maps.append(
            {
                "w": w_pad,
                "cjb": cjb,
                "gidx": gidx,
                "dstloc": dstloc,
                "cib": cib,
            }
        )
    return in_maps, k_s


def _maybe_enable_ldw_opt():
    if not int(os.environ.get("KERNEL_LDW", "0")):
        return
    import concourse.bass_utils as _bu

    if getattr(_bu, "_ldw_patched", False):
        return
    _orig = _bu.run_command

    def _patched(argv, **kw):
        argv = [
            "--enable-ldw-opt=true" if a == "--enable-ldw-opt=false" else a
            for a in argv
        ]
        return _orig(argv, **kw)

    _bu.run_command = _patched
    _bu._ldw_patched = True


def kernel(weight, cj, ci, src, dst):
    global LAST_EXEC_NS
    _maybe_enable_ldw_opt()
    weight = np.asarray(weight, dtype=np.float32)
    cj = np.asarray(cj, dtype=np.float32)
    ci = np.asarray(ci, dtype=np.float32)
    src = np.asarray(src, dtype=np.int32)
    dst = np.asarray(dst, dtype=np.int32)

    in_maps, k_s = _prep_inputs(weight, cj, ci, src, dst)
    nc = _build_program(k_s)
    nc.finalize()
    trace = bool(int(os.environ.get("KERNEL_TRACE", "0")))
    if trace:
        _ensure_ntff_hook()
    try:
        res = run_bass_kernel_spmd(
            nc, in_maps, core_ids=list(range(N_CORES)), trace=trace
        )
    except Exception:
        if not trace:
            raise
        res = run_bass_kernel_spmd(
            nc, in_maps, core_ids=list(range(N_CORES)), trace=False
        )
    LAST_EXEC_NS = res.exec_time_ns
    out = np.concatenate(
        [res.results[c]["h"][:DST_PER_CORE] for c in range(N_CORES)], axis=0
    )
    return out.astype(np.float32)


# revision 19
# speedup vs baseline: 1.1092x; 1.0731x over previous
"""GCMCGraphConv Bass kernel for 8 TRN2 NeuronCores.

Computes: h = ci * segment_sum((weight * cj)[src], dst)  for a random
graph with N=100000 nodes, F=128 features, E=1600000 edges.

Strategy (1D dst-partitioning, v4):
  - core c owns dst rows [c*12500, (c+1)*12500)
  - host partitions edges by dst owner, groups by (dst block, src
    segment), pads each (block, segment) run to K_s chunks of 128
    edges (uniform across blocks/cores so the SPMD program is static)
  - device phase 1 (prescale): wsc = bf16(weight * cj), written to 4
    internal DRAM segments of 25600 rows (so gather indices fit int16)
  - device phase 2: per src segment the edge-source rows are fetched
    by dma_gather ucode instructions of 1024 indices (8 chunks) each;
    per (block, segment) a batched is_equal one-hot of dst_local
    (ACT/DVE) and per chunk a bf16 matmul accumulate the segment sum
    in PSUM; ci scaling and an output DMA finish each block
"""

import os
import sys

import numpy as np

sys.path.insert(0, "/opt/trn_rl_repo")

from concourse import bacc, bass, mybir  # noqa: E402
import concourse.tile as tile  # noqa: E402
from concourse.bass_utils import run_bass_kernel_spmd  # noqa: E402

N_NODES = 100000
FEAT = 128
N_CORES = 8
DST_PER_CORE = N_NODES // N_CORES  # 12500
P = 128
N_BLOCKS = (DST_PER_CORE + P - 1) // P  # 98
DST_PAD = N_BLOCKS * P  # 12544

SEG = 4
SEG_ROWS = 25600  # multiple of SUPER; int16-addressable
N_PAD = SEG * SEG_ROWS  # 102400
SUPER = 1024  # convert-pass superblock rows
PIECE = 8  # chunks per dma_gather instruction (1024 idx ring limit)

LAST_EXEC_NS = None


def _ensure_ntff_hook():
    """Shim antenv.axon_hooks if the image's antenv predates it."""
    import types

    try:
        from antenv.axon_hooks import get_axon_ntff_profile_hook  # noqa: F401

        return
    except ImportError:
        pass
    try:
        import antenv

        mod = types.ModuleType("antenv.axon_hooks")
        _hook = [None]
        mod.set_axon_ntff_profile_hook = lambda h: _hook.__setitem__(0, h)
        mod.get_axon_ntff_profile_hook = lambda: _hook[0]
        antenv.axon_hooks = mod
        sys.modules["antenv.axon_hooks"] = mod
        from trn_agent_boot.trn_boot import _ntff_profile_via_ctypes

        mod.set_axon_ntff_profile_hook(
            _ntff_profile_via_ctypes("/opt/axon/libaxon_pjrt.so")
        )
    except Exception:
        pass


def _build_program(k_s: int) -> bass.Bass:
    """One SPMD program; every core runs it on its own edge shard."""
    nc = bacc.Bacc(num_swdge_queues=4)
    f32 = mybir.dt.float32
    bf16 = mybir.dt.bfloat16
    i32 = mybir.dt.int32
    i16 = mybir.dt.int16

    n_super = N_PAD // SUPER
    blk_chunks = SEG * k_s  # chunks per dst block
    ncols = N_BLOCKS * blk_chunks
    seg_chunks = N_BLOCKS * k_s  # chunks per segment
    n_pieces = (seg_chunks + PIECE - 1) // PIECE
    idxcols_per_piece = PIECE * P // 16  # 64

    w_d = nc.declare_dram_parameter("w", [N_PAD, FEAT], f32, isOutput=False)
    cjb_d = nc.declare_dram_parameter("cjb", [P, N_PAD // P], f32, isOutput=False)
    gidx_d = nc.declare_dram_parameter(
        "gidx", [P, SEG * n_pieces * idxcols_per_piece], i16, isOutput=False
    )
    dstloc_d = nc.declare_dram_parameter("dstloc", [P, ncols], bf16, isOutput=False)
    cib_d = nc.declare_dram_parameter("cib", [P, N_BLOCKS], f32, isOutput=False)
    h_d = nc.declare_dram_parameter("h", [DST_PAD, FEAT], f32, isOutput=True)

    with tile.TileContext(nc) as tc:
        with (
            tc.tile_pool(name="wscp0", bufs=1, space="DRAM") as wscp0,
            tc.tile_pool(name="wscp1", bufs=1, space="DRAM") as wscp1,
            tc.tile_pool(name="wscp2", bufs=1, space="DRAM") as wscp2,
            tc.tile_pool(name="wscp3", bufs=1, space="DRAM") as wscp3,
            tc.tile_pool(name="meta", bufs=1) as meta,
            tc.tile_pool(name="conv", bufs=3) as conv,
            tc.tile_pool(name="gather", bufs=5) as gpool,
            tc.tile_pool(name="work", bufs=4) as work,
            tc.tile_pool(name="out", bufs=3) as opool,
            tc.tile_pool(name="psum", bufs=2, space="PSUM") as psum,
        ):
            wsc = [
                pool.tile([SEG_ROWS, FEAT], bf16, tag=f"wsc{s}", name=f"wsc{s}")
                for s, pool in enumerate([wscp0, wscp1, wscp2, wscp3])
            ]

            gidx = meta.tile([P, SEG * n_pieces * idxcols_per_piece], i16)
            dstloc = meta.tile([P, ncols], bf16)
            cib = meta.tile([P, N_BLOCKS], f32)
            cjb = meta.tile([P, N_PAD // P], f32)
            nc.sync.dma_start(out=gidx[:], in_=gidx_d[:])
            nc.sync.dma_start(out=dstloc[:], in_=dstloc_d[:])
            nc.sync.dma_start(out=cib[:], in_=cib_d[:])
            nc.sync.dma_start(out=cjb[:], in_=cjb_d[:])

            # iota5[p, j*128 + f] = f  (int32)
            iota5i = meta.tile([P, k_s * P], i32)
            nc.gpsimd.iota(
                iota5i[:], pattern=[[0, k_s], [1, P]], base=0, channel_multiplier=0
            )
            iota5 = meta.tile([P, k_s * P], bf16)
            nc.vector.tensor_copy(out=iota5[:], in_=iota5i[:])

            # phase 1: wsc[seg] = bf16(w * cj), 1024-row superblocks
            spb = SUPER // P  # 8 column-groups per superblock
            sbs_per_seg = SEG_ROWS // SUPER
            for sb in range(n_super):
                wt = conv.tile([P, SUPER], f32, tag="wt")
                win = w_d[sb * SUPER : (sb + 1) * SUPER, :].rearrange(
                    "(g p) f -> p g f", p=P
                )
                nc.sync.dma_start(
                    out=wt[:].rearrange("p (g f) -> p g f", f=FEAT), in_=win
                )
                ws = conv.tile([P, SUPER], bf16, tag="ws")
                nc.vector.tensor_tensor(
                    out=ws[:].rearrange("p (g f) -> p g f", f=FEAT),
                    in0=wt[:].rearrange("p (g f) -> p g f", f=FEAT),
                    in1=cjb[:, sb * spb : (sb + 1) * spb].to_broadcast([P, spb, FEAT]),
                    op=mybir.AluOpType.mult,
                )
                s = sb // sbs_per_seg
                lb = sb % sbs_per_seg
                wout = wsc[s][lb * SUPER : (lb + 1) * SUPER, :].rearrange(
                    "(g p) f -> p g f", p=P
                )
                nc.scalar.dma_start(
                    out=wout, in_=ws[:].rearrange("p (g f) -> p g f", f=FEAT)
                )

            # phase 2: issue all gathers (Tile paces them via pool bufs),
            # then per block: one-hot + matmul chunks, ci scale, store.
            gts: list[dict] = [{} for _ in range(SEG)]
            issue_order = [(s, pc) for s in range(SEG) for pc in range(5)] + [
                (s, pc) for pc in range(5, n_pieces) for s in range(SEG)
            ]
            for s, pc in issue_order:
                if True:
                    nchunk = min(PIECE, seg_chunks - pc * PIECE)
                    gt = gpool.tile([P, PIECE * FEAT], bf16, tag=f"gw{s}")
                    inst = s * n_pieces + pc
                    nc.gpsimd.dma_gather(
                        gt[:, : nchunk * FEAT].rearrange("p (m f) -> p m f", f=FEAT),
                        wsc[s][:],
                        gidx[
                            :,
                            inst * idxcols_per_piece : inst * idxcols_per_piece
                            + nchunk * P // 16,
                        ],
                        nchunk * P,
                        nchunk * P,
                        FEAT,
                        queue_num=s,
                    )
                    gts[s][pc] = gt

            for b in range(N_BLOCKS):
                acc = psum.tile([P, FEAT], f32, tag="acc")
                ci_i = 0
                for s in range(SEG):
                    onehot = work.tile([P, k_s * P], bf16, tag="onehot")
                    nc.any.tensor_tensor(
                        out=onehot[:].rearrange("p (m f) -> p m f", f=P),
                        in0=dstloc[
                            :, b * blk_chunks + s * k_s : b * blk_chunks + (s + 1) * k_s
                        ].to_broadcast([P, k_s, P]),
                        in1=iota5[:].rearrange("p (m f) -> p m f", f=P),
                        op=mybir.AluOpType.is_equal,
                    )
                    for k in range(k_s):
                        q = b * k_s + k  # global chunk index within segment
                        gt = gts[s][q // PIECE]
                        off = q % PIECE
                        nc.tensor.matmul(
                            out=acc[:],
                            lhsT=onehot[:, k * P : (k + 1) * P],
                            rhs=gt[:, off * FEAT : (off + 1) * FEAT],
                            start=(ci_i == 0),
                            stop=(ci_i == blk_chunks - 1),
                        )
                        ci_i += 1
                ho = opool.tile([P, FEAT], f32, tag="ho")
                nc.vector.tensor_tensor(
                    out=ho[:],
                    in0=acc[:],
                    in1=cib[:, b : b + 1].to_broadcast([P, FEAT]),
                    op=mybir.AluOpType.mult,
                )
                nc.sync.dma_start(out=h_d[b * P : (b + 1) * P, :], in_=ho[:])
    return nc


def _prep_inputs(weight, cj, ci, src, dst):
    """Partition edges by dst owner; build per-core metadata arrays."""
    order = np.argsort(dst, kind="stable")
    ds = dst[order].astype(np.int64)
    ss = src[order].astype(np.int64)
    core_bounds = np.searchsorted(ds, np.arange(N_CORES + 1) * DST_PER_CORE)

    cores = []
    k_s = 1
    for c in range(N_CORES):
        a, b = core_bounds[c], core_bounds[c + 1]
        d_local = ds[a:b] - c * DST_PER_CORE
        s_c = ss[a:b]
        seg = s_c // SEG_ROWS
        block = d_local // P
        o2 = np.lexsort((seg, block))
        d_local, s_c, seg, block = d_local[o2], s_c[o2], seg[o2], block[o2]
        bs = block * SEG + seg
        counts = np.bincount(bs, minlength=N_BLOCKS * SEG)
        k_s = max(k_s, int(np.ceil(counts.max() / P)))
        cores.append((d_local, s_c, bs, counts))

    blk_chunks = SEG * k_s
    ncols = N_BLOCKS * blk_chunks
    seg_chunks = N_BLOCKS * k_s
    n_pieces = (seg_chunks + PIECE - 1) // PIECE
    idxcols_per_piece = PIECE * P // 16

    cj_flat = cj.reshape(-1).astype(np.float32)
    ci_flat = ci.reshape(-1).astype(np.float32)

    w_pad = np.zeros((N_PAD, FEAT), dtype=np.float32)
    w_pad[:N_NODES] = weight
    cj_pad = np.zeros(N_PAD, dtype=np.float32)
    cj_pad[:N_NODES] = cj_flat
    cjb = cj_pad.reshape(N_PAD // P, P).T.copy()

    in_maps = []
    for c in range(N_CORES):
        d_local, s_c, bs, counts = cores[c]
        starts = np.zeros(N_BLOCKS * SEG, dtype=np.int64)
        starts[1:] = np.cumsum(counts)[:-1]
        wbi = np.arange(len(d_local)) - starts[bs]  # index within (block, seg) run
        kk = wbi // P
        pp = wbi % P
        col = (bs // SEG) * blk_chunks + (bs % SEG) * k_s + kk

        import ml_dtypes

        dstloc = np.full((P, ncols), -1, dtype=ml_dtypes.bfloat16)
        dstloc[pp, col] = (d_local % P).astype(ml_dtypes.bfloat16)
        srcloc = np.zeros((P, ncols), dtype=np.int16)
        srcloc[pp, col] = (s_c % SEG_ROWS).astype(np.int16)

        # gather index arrays: per (seg, piece) instruction, idx j at
        # [16*grp + j%16, j//16]; j = (chunk_within_piece*128 + p),
        # chunk q (= b*k_s + k) of segment s is piece q//PIECE.
        gidx = np.zeros((P, SEG * n_pieces * idxcols_per_piece), dtype=np.int16)
        for s in range(SEG):
            # [P, seg_chunks] source-local indices for this segment in
            # chunk order q = b*k_s + k  -> col = b*blk_chunks + s*k_s + k
            cols = (
                (np.arange(N_BLOCKS)[:, None] * blk_chunks)
                + s * k_s
                + np.arange(k_s)[None, :]
            ).reshape(-1)
            segsrc = srcloc[:, cols]  # [P, seg_chunks]
            vals = segsrc.T.reshape(-1)  # j = q*128 + p
            vals = np.pad(vals, (0, n_pieces * PIECE * P - len(vals)))
            block16 = vals.reshape(n_pieces * idxcols_per_piece, 16).T  # [16, cols]
            gidx[
                :, s * n_pieces * idxcols_per_piece : (s + 1) * n_pieces * idxcols_per_piece
            ] = np.tile(block16, (8, 1))

        ci_pad = np.zeros(DST_PAD, dtype=np.float32)
        ci_pad[:DST_PER_CORE] = ci_flat[c * DST_PER_CORE : (c + 1) * DST_PER_CORE]
        cib = ci_pad.reshape(N_BLOCKS, P).T.copy()

        in_maps.append(
            {
                "w": w_pad,
                "cjb": cjb,
                "gidx": gidx,
                "dstloc": dstloc,
                "cib": cib,
            }
        )
    return in_maps, k_s


def _maybe_enable_ldw_opt():
    if not int(os.environ.get("KERNEL_LDW", "0")):
        return
    import concourse.bass_utils as _bu

    if getattr(_bu, "_ldw_patched", False):
        return
    _orig = _bu.run_command

    def _patched(argv, **kw):
        argv = [
            "--enable-ldw-opt=true" if a == "--enable-ldw-opt=false" else a
            for a in argv
        ]
        return _orig(argv, **kw)

    _bu.run_command = _patched
    _bu._ldw_patched = True


def kernel(weight, cj, ci, src, dst):
    global LAST_EXEC_NS
    _maybe_enable_ldw_opt()
    weight = np.asarray(weight, dtype=np.float32)
    cj = np.asarray(cj, dtype=np.float32)
    ci = np.asarray(ci, dtype=np.float32)
    src = np.asarray(src, dtype=np.int32)
    dst = np.asarray(dst, dtype=np.int32)

    in_maps, k_s = _prep_inputs(weight, cj, ci, src, dst)
    nc = _build_program(k_s)
    nc.finalize()
    trace = bool(int(os.environ.get("KERNEL_TRACE", "0")))
    if trace:
        _ensure_ntff_hook()
    try:
        res = run_bass_kernel_spmd(
            nc, in_maps, core_ids=list(range(N_CORES)), trace=trace
        )
    except Exception:
        if not trace:
            raise
        res = run_bass_kernel_spmd(
            nc, in_maps, core_ids=list(range(N_CORES)), trace=False
        )
    LAST_EXEC_NS = res.exec_time_ns
    out = np.concatenate(
        [res.results[c]["h"][:DST_PER_CORE] for c in range(N_CORES)], axis=0
    )
    return out.astype(np.float32)
